# revision 8
# baseline (speedup 1.0000x reference)
"""Trainium2 Bass kernel for the 3-layer LSTM (nn_AttLSTM_8899172237963).

Measured constraints of this execution environment drove the design:
- Inter-core exchange is unusable per-step (remote DMA descriptors fault;
  a collective costs ~240us), so the serial T=512 recurrence runs on ONE
  core.
- DMA sustains only ~1-12 GB/s here, so weights cannot be streamed per
  step. The kernel runs LAYER-SEQUENTIAL PHASES: phase l keeps only
  layer l's weights resident in SBUF (~16.8MB bf16) and runs all 512
  timesteps of that layer; h hands off between phases through DRAM
  (transposed, bf16, ~128KB/step).

Per step (phase l): pre = [h_l, in] @ W_l + b accumulates in PSUM in a
"doubled" layout (partitions 0:64 = batch x A-half gate columns,
64:128 = batch x B-half) via paired col-group matmuls (tile_position
(0,0)/(0,64)). Gates run fp32 on ACT/DVE; c is fp32, h bf16. hT for the
next step comes from PE transposes + two ACT copies (one partition-
shifting), and is also the DRAM handoff payload.
"""
import numpy as np
import ml_dtypes

import concourse.bass as bass
import concourse.mybir as mybir
import concourse.tile as tile
from concourse import bacc
from concourse.bass_utils import run_bass_kernel_spmd
from concourse.masks import make_identity
from concourse.bass import ds

BF = ml_dtypes.bfloat16
f32 = mybir.dt.float32
bf16 = mybir.dt.bfloat16
AL = mybir.AluOpType
AF = mybir.ActivationFunctionType

B, T, D, H, C = 64, 512, 256, 1024, 128
KEEP = float(np.float32(0.9))
DROP = float(np.float32(1.0) - np.float32(0.9))


# ---------------------------------------------------------------- host pack
def _gate_cols():
    colsA, colsB = [], []
    for j in range(8):
        a = np.concatenate([g * H + 128 * j + np.arange(64) for g in range(4)])
        colsA.append(a)
        colsB.append(a + 64)
    return np.concatenate(colsA), np.concatenate(colsB)


def _pack_w(Wpart, cols):
    kc = Wpart.shape[0] // 128
    w = Wpart[:, cols].reshape(kc, 128, 2048)
    return np.ascontiguousarray(np.transpose(w, (1, 0, 2))).astype(BF)


def pack_inputs(inputs):
    x = np.asarray(inputs["struct"], np.float32)
    colsA, colsB = _gate_cols()
    m = {}
    xT = np.transpose(x, (2, 1, 0)).reshape(2, 128, T * B)
    m["xT"] = np.ascontiguousarray(np.transpose(xT, (1, 0, 2))).astype(BF)
    ball = []
    for l in (1, 2, 3):
        W = np.asarray(inputs[f"W{l}"], np.float32)
        b = np.asarray(inputs[f"b{l}"], np.float32)
        m[f"WhA{l}"] = _pack_w(W[:H], colsA)
        m[f"WhB{l}"] = _pack_w(W[:H], colsB)
        m[f"WxA{l}"] = _pack_w(W[H:], colsA)
        m[f"WxB{l}"] = _pack_w(W[H:], colsB)
        ball += [b[colsA], b[colsB]]
    m["bias_all"] = np.concatenate(ball)[None, :].astype(BF)
    fcw = np.asarray(inputs["fc_w"], np.float32)
    fcwT = fcw.T.reshape(8, 128, C)
    m["fcwT"] = np.ascontiguousarray(np.transpose(fcwT, (1, 0, 2))).astype(BF)
    m["fcb"] = np.asarray(inputs["fc_b"], np.float32)[None, :].astype(BF)
    return m


# ---------------------------------------------------------------- kernel IR
class _KB:
    pass


def build(nsteps=T):
    nc = bacc.Bacc("TRN2", target_bir_lowering=False, debug=False,
                   enable_asserts=False, num_devices=1)
    kb = _KB()
    kb.nc = nc
    kb.nsteps = nsteps
    din = {}
    din["xT"] = nc.dram_tensor("xT", [128, 2, nsteps * B], bf16, kind="ExternalInput")
    for l in (1, 2, 3):
        kx = 2 if l == 1 else 8
        for g in "AB":
            din[f"Wh{g}{l}"] = nc.dram_tensor(f"Wh{g}{l}", [128, 8, 2048], bf16,
                                              kind="ExternalInput")
            din[f"Wx{g}{l}"] = nc.dram_tensor(f"Wx{g}{l}", [128, kx, 2048], bf16,
                                              kind="ExternalInput")
    din["bias_all"] = nc.dram_tensor("bias_all", [1, 6 * 2048], bf16,
                                     kind="ExternalInput")
    din["fcwT"] = nc.dram_tensor("fcwT", [128, 8, C], bf16, kind="ExternalInput")
    din["fcb"] = nc.dram_tensor("fcb", [1, C], bf16, kind="ExternalInput")
    out_d = nc.dram_tensor("out", [B, C], f32, kind="ExternalOutput")
    kb.din = din
    kb.out_ap = out_d.ap()
    kb.hdram = {
        1: nc.dram_tensor("h1d", [128, nsteps, 8, 64], bf16, kind="Internal"),
        2: nc.dram_tensor("h2d", [128, nsteps, 8, 64], bf16, kind="Internal"),
    }

    with tile.TileContext(nc) as tc:
        kb.tc = tc
        with tc.tile_pool(name="const", bufs=1) as cpool, \
             tc.tile_pool(name="wts", bufs=1) as wtpool, \
             tc.tile_pool(name="stream", bufs=4) as spool, \
             tc.tile_pool(name="work", bufs=2) as wpool, \
             tc.tile_pool(name="psum", bufs=6, space="PSUM") as ppool, \
             tc.tile_pool(name="psumt", bufs=2, space="PSUM") as ptpool:
            kb.cpool, kb.wtpool, kb.spool, kb.wpool = cpool, wtpool, spool, wpool
            kb.ppool, kb.ptpool = ppool, ptpool

            kb.ident = cpool.tile([128, 128], bf16)
            make_identity(nc, kb.ident[:])
            kb.ones = cpool.tile([1, 64], bf16)
            nc.vector.memset(kb.ones[:], 1.0)
            kb.bias_all = cpool.tile([1, 6 * 2048], bf16, name="bias_all")
            nc.sync.dma_start(kb.bias_all[:], din["bias_all"].ap())
            kb.bias = {}
            for li, l in enumerate((1, 2, 3)):
                for gi, g in enumerate("AB"):
                    off = (li * 2 + gi) * 2048
                    kb.bias[(g, l)] = kb.bias_all[0:1, off:off + 2048]
            kb.fcw = cpool.tile([128, 8, C], bf16)
            nc.sync.dma_start(kb.fcw[:], din["fcwT"].ap())
            kb.fcb = cpool.tile([1, C], bf16)
            nc.sync.dma_start(kb.fcb[:], din["fcb"].ap())

            # state tiles (reused across phases)
            kb.c_st = cpool.tile([128, 8, 64], f32, name="c_st")
            kb.h_st = cpool.tile([128, 8, 64], bf16, name="h_st")
            kb.hT = [cpool.tile([128, 8, 64], bf16, name=f"hT{p}") for p in range(2)]

            for l in (1, 2, 3):
                _emit_phase(kb, l)
            _emit_fc(kb)
    nc.compile()
    return nc


def _emit_phase(kb, l):
    nc, tc, nsteps = kb.nc, kb.tc, kb.nsteps
    kx = 2 if l == 1 else 8

    W = {}
    for g in "AB":
        wh = kb.wtpool.tile([128, 8, 2048], bf16, tag=f"Wh{g}", name=f"pWh{g}{l}")
        nc.sync.dma_start(wh[:], kb.din[f"Wh{g}{l}"].ap())
        wx = kb.wtpool.tile([128, kx, 2048], bf16, tag=f"Wx{g}", name=f"pWx{g}{l}")
        nc.sync.dma_start(wx[:], kb.din[f"Wx{g}{l}"].ap())
        W[("h", g)] = wh
        W[("x", g)] = wx

    nc.vector.memset(kb.c_st[:], 0)
    nc.vector.memset(kb.h_st[:], 0)
    nc.vector.memset(kb.hT[0][:], 0)
    nc.vector.memset(kb.hT[1][:], 0)

    _emit_phase_step(kb, l, W, 0, static=True)
    n_pairs = (nsteps - 1) // 2
    with tc.For_i(0, n_pairs, 1) as i:
        _emit_phase_step(kb, l, W, 1, static=False, ivar=i)
        _emit_phase_step(kb, l, W, 2, static=False, ivar=i)
    for s in range(1 + 2 * n_pairs, nsteps):
        _emit_phase_step(kb, l, W, s, static=True)


def _emit_phase_step(kb, l, W, s, static, ivar=None):
    """One timestep of phase l; time index t = s (static) or s + 2*ivar."""
    nc = kb.nc
    par, nxt = s % 2, (s + 1) % 2
    kx = 2 if l == 1 else 8

    if l == 1:
        xch = kb.spool.tile([128, 2, 64], bf16, tag="xT")
        if static:
            nc.sync.dma_start(xch[:], kb.din["xT"].ap()[:, :, s * B:(s + 1) * B])
        else:
            nc.sync.dma_start(xch[:],
                              kb.din["xT"].ap()[:, :, ds(ivar * (2 * B) + s * B, B)])
        in_ch = [xch[:, kc, :] for kc in range(2)]
    else:
        hp = kb.spool.tile([128, 8, 64], bf16, tag="hprev")
        src = kb.hdram[l - 1].ap()
        if static:
            nc.sync.dma_start(hp[:], src[:, s, :, :])
        else:
            nc.sync.dma_start(hp[:], src[:, ds(ivar * 2 + s, 1), :, :])
        in_ch = [hp[:, kc, :] for kc in range(8)]

    rec_active = (not static) or (s > 0)
    hT = kb.hT[par]

    psums = []
    for ck in range(4):
        ps = kb.ppool.tile([128, 512], f32, tag="pre", name="pre")
        psums.append(ps)
        cols = slice(512 * ck, 512 * ck + 512)
        ops = [(W[("x", "A")][:, kc, cols], W[("x", "B")][:, kc, cols], in_ch[kc])
               for kc in range(kx)]
        if rec_active:
            ops += [(W[("h", "A")][:, kc, cols], W[("h", "B")][:, kc, cols],
                     hT[:, kc, :]) for kc in range(8)]
        for oi, (wa, wb, lhs) in enumerate(ops):
            nc.tensor.matmul(ps[0:64, :], lhs, wa, start=(oi == 0), stop=False,
                             tile_position=(0, 0), skip_group_check=True)
            nc.tensor.matmul(ps[64:128, :], lhs, wb, start=(oi == 0), stop=False,
                             tile_position=(0, 64), skip_group_check=True)
        nc.tensor.matmul(ps[0:64, :], kb.ones[:], kb.bias[("A", l)][:, cols],
                         start=False, stop=True,
                         tile_position=(0, 0), skip_group_check=True)
        nc.tensor.matmul(ps[64:128, :], kb.ones[:], kb.bias[("B", l)][:, cols],
                         start=False, stop=True,
                         tile_position=(0, 64), skip_group_check=True)

    # gates (fp32 intermediates)
    c_st, h_st = kb.c_st, kb.h_st
    sf = kb.wpool.tile([128, 8, 64], f32, tag="sf")
    si = kb.wpool.tile([128, 8, 64], f32, tag="si")
    so = kb.wpool.tile([128, 8, 64], f32, tag="so")
    tg = kb.wpool.tile([128, 8, 64], f32, tag="tg")
    for ck in range(4):
        psv = psums[ck][:].rearrange("p (sl f) -> p sl f", sl=2)
        s0 = 2 * ck
        for gate, dst in ((0, sf), (1, si), (2, so), (3, tg)):
            fn = AF.Tanh if gate == 3 else AF.Sigmoid
            nc.scalar.activation(dst[:, s0:s0 + 2, :],
                                 psv[:, :, gate * 64:(gate + 1) * 64], fn)
    # c_raw = sf*c + si*tg ; h_new = KEEP*so*tanh(c_raw) + DROP*h
    # c_new = KEEP*c_raw + DROP*c   (reference order: tanh sees pre-blend c)
    t1 = kb.wpool.tile([128, 8, 64], f32, tag="t1", bufs=1)
    nc.vector.tensor_tensor(t1[:], si[:], tg[:], AL.mult)
    t2 = kb.wpool.tile([128, 8, 64], f32, tag="t2", bufs=1)
    nc.vector.tensor_tensor(t2[:], sf[:], c_st[:], AL.mult)
    craw = kb.wpool.tile([128, 8, 64], f32, tag="craw", bufs=1)
    nc.vector.tensor_add(craw[:], t1[:], t2[:])
    tc_ = kb.wpool.tile([128, 8, 64], f32, tag="tc", bufs=1)
    nc.scalar.activation(tc_[:], craw[:], AF.Tanh)
    t5 = kb.wpool.tile([128, 8, 64], f32, tag="t5", bufs=1)
    nc.vector.tensor_scalar_mul(t5[:], c_st[:], DROP)
    nc.vector.scalar_tensor_tensor(c_st[:], craw[:], KEEP, t5[:], AL.mult, AL.add)
    t4 = kb.wpool.tile([128, 8, 64], f32, tag="t4", bufs=1)
    nc.vector.scalar_tensor_tensor(t4[:], so[:], KEEP, tc_[:], AL.mult, AL.mult)
    nc.vector.scalar_tensor_tensor(h_st[:], h_st[:], DROP, t4[:], AL.mult, AL.add)

    # transposes -> hT[nxt] (+ DRAM handoff for l < 3)
    pt = kb.ptpool.tile([64, 8, 128], bf16, tag="pt")
    for j in range(8):
        nc.tensor.transpose(pt[:, j, :], h_st[:, j, :], kb.ident[:])
    dstT = kb.hT[nxt]
    nc.scalar.copy(dstT[0:64, :, :], pt[:, :, 0:64])
    nc.scalar.copy(dstT[64:128, :, :], pt[:, :, 64:128])
    if l < 3:
        dst = kb.hdram[l].ap()
        if static:
            nc.sync.dma_start(dst[:, s, :, :], dstT[:])
        else:
            nc.sync.dma_start(dst[:, ds(ivar * 2 + s, 1), :, :], dstT[:])


def _emit_fc(kb):
    nc = kb.nc
    par = kb.nsteps % 2
    h3T = kb.hT[par]
    ps = kb.ppool.tile([64, C], f32, tag="pre", name="fcps")
    for kc in range(8):
        nc.tensor.matmul(ps[:, :], h3T[:, kc, :], kb.fcw[:, kc, :],
                         start=(kc == 0), stop=False, skip_group_check=True)
    nc.tensor.matmul(ps[:, :], kb.ones[:], kb.fcb[:],
                     start=False, stop=True, skip_group_check=True)
    r1 = kb.wpool.tile([64, C], f32, tag="r1")
    nc.scalar.activation(r1[:], ps[:], AF.Relu)
    xm = kb.wpool.tile([64, C], f32, tag="xm")
    nc.vector.tensor_scalar_min(xm[:], ps[:], 0.0)
    e1 = kb.wpool.tile([64, C], f32, tag="e1")
    nc.scalar.activation(e1[:], xm[:], AF.Exp)
    e2 = kb.wpool.tile([64, C], f32, tag="e2")
    nc.vector.tensor_scalar_add(e2[:], e1[:], -1.0)
    o = kb.wpool.tile([64, C], f32, tag="o")
    nc.vector.tensor_add(o[:], r1[:], e2[:])
    nc.sync.dma_start(kb.out_ap, o[:])


_CACHED = {}


def kernel(**inputs):
    m = pack_inputs(inputs)
    if "nc" not in _CACHED:
        _CACHED["nc"] = build(T)
    nc = _CACHED["nc"]
    res = run_bass_kernel_spmd(nc, [m], core_ids=[0])
    return np.asarray(res.results[0]["out"], np.float32)
